# revision 14
# baseline (speedup 1.0000x reference)
"""Causal self-attention Bass/Tile kernel for Trainium2 (8 NeuronCores).

Problem: y = CausalSelfAttention(x) with
  B=8, T=1024, C=1024, H=16 heads, hs=64.
  qkv = x @ W_qkv + b_qkv;  per-head causal softmax(q k^T / sqrt(hs)) @ v;
  y = out @ W_proj + b_proj.

Sharding: pure data parallel - core i computes batch element i end-to-end.
No collectives.

v2 design notes (all matmuls bf16, PE issue rate ~0.44ns/col at 2.4GHz):
  - x is transposed AND cast to bf16 on the host; xT [C,T] DMAs straight
    into SBUF. No PE transposes, no identity matrix.
  - qkT [2C,T] = (W_qk)^T x^T (lhsT = W chunk, rhs = xT), score scale
    pre-folded into W_q/b_q. PSUM->SBUF copy + bias runs on the SCALAR
    engine (ACT Identity with per-partition bias AP) to keep DVE free.
  - v [T,C] natural (lhsT = xT chunk, rhs = W_v), bf16 into
    v_pad [T, kb, h, 65] whose 65th column is ones (fused row-sum).
    PSUM->SBUF + bias runs on the POOL engine (otherwise idle).
  - Scores TRANSPOSED: S^T[k,q] = matmul(lhsT=kT chunk, rhs=qT), two heads
    packed on PE row-groups (K=64) via tile_position. Score chunks are
    512-ALIGNED on the q grid so each chunk maps 1:1 to a PV matmul.
    exp on ACT straight out of single-bank PSUM tiles (4 in flight),
    writing a PAIRED pt tile [P, 2, w]; one DVE mask-mul covers both
    heads' diagonal blocks.
  - PV: outT[h] [65,q] += matmul(lhsT=v_pad[:,kb,h,:], rhs=P^T chunk).
    Row 64 = softmax denominator. Normalize: DVE reciprocal of the psum
    row -> [1,512], POOL partition_broadcast -> [64,512], DVE multiply
    during the PSUM->SBUF copy of outT.
  - proj: y [T,C] = matmul(lhsT=outT chunk, rhs=W_proj) + b_proj (DVE).
  - Emission interleaves qkT/v gemm "filler" units between score chunks
    so the PE never stalls on the score->exp->PV chain and the HAM
    clock-gate never re-throttles it.
"""

import os
from contextlib import ExitStack

import numpy as np
import ml_dtypes

import concourse.bass as bass
import concourse.bacc as bacc
import concourse.mybir as mybir
import concourse.tile as tile
from concourse.bass_utils import run_bass_kernel_spmd

F32 = mybir.dt.float32
BF16 = mybir.dt.bfloat16
AF = mybir.ActivationFunctionType

P = 128
B = 8
T = 1024
C = 1024
H = 16
HS = 64
TO = T // P   # 8 t-blocks
CO = C // P   # 8 c-chunks
NPAIR = H // 2  # 8 head pairs

# 512-aligned score/PV chunks per key block: list of (qstart, width)
CHUNKS = {}
for _kb in range(TO):
    _lst = []
    _s = _kb * P
    while _s < T:
        _e = min((_s // 512 + 1) * 512, T)
        _lst.append((_s, _e - _s))
        _s = _e
    CHUNKS[_kb] = _lst

# module-level knobs for test.py
TRACE = bool(int(os.environ.get("KERNEL_TRACE", "0")))
DEBUG = bool(int(os.environ.get("KERNEL_DEBUG", "0")))
LAST_RESULTS = None  # BassKernelResults of last run


def build_nc():
    nc = bacc.Bacc("TRN2", target_bir_lowering=False, debug=False)

    xT_d = nc.dram_tensor("xT", [C, T], BF16, kind="ExternalInput").ap()
    wqk_d = nc.dram_tensor("wqk", [C, 2 * C], BF16, kind="ExternalInput").ap()
    wv_d = nc.dram_tensor("wv", [C, C], BF16, kind="ExternalInput").ap()
    wproj_d = nc.dram_tensor("wproj", [C, C], BF16, kind="ExternalInput").ap()
    bqk_d = nc.dram_tensor("bqk", [2 * C], F32, kind="ExternalInput").ap()
    bv_d = nc.dram_tensor("bv", [C], F32, kind="ExternalInput").ap()
    bproj_d = nc.dram_tensor("bproj", [C], F32, kind="ExternalInput").ap()
    mask2_d = nc.dram_tensor("mask2", [P, 2 * P], BF16, kind="ExternalInput").ap()
    y_d = nc.dram_tensor("y", [T, C], F32, kind="ExternalOutput").ap()
    dbg = None
    if DEBUG:
        dbg = dict(
            qkT=nc.dram_tensor("dbg_qkT", [P, 2 * C // P, T], BF16,
                               kind="ExternalOutput").ap(),
            vpad=nc.dram_tensor("dbg_vpad", [P, TO, H, HS + 1], BF16,
                                kind="ExternalOutput").ap(),
            pt=nc.dram_tensor("dbg_pt", [P, 2, 4608], BF16,
                              kind="ExternalOutput").ap(),
            brow=nc.dram_tensor("dbg_brow", [HS, 2, 2, 512], F32,
                                kind="ExternalOutput").ap(),
            outT=nc.dram_tensor("dbg_outT", [P, CO, T], BF16,
                                kind="ExternalOutput").ap(),
        )

    with tile.TileContext(nc) as tc:
        _attn_body(tc, xT_d, wqk_d, wv_d, wproj_d, bqk_d, bv_d, bproj_d,
                   mask2_d, y_d, dbg)
    nc.compile()
    return nc


def _attn_body(tc, xT_d, wqk_d, wv_d, wproj_d, bqk_d, bv_d, bproj_d,
               mask2_d, y_d, dbg=None):
    nc = tc.nc
    with ExitStack() as ctx:
        consts = ctx.enter_context(tc.tile_pool(name="consts", bufs=1))
        big = ctx.enter_context(tc.tile_pool(name="big", bufs=1))
        ps_mm = ctx.enter_context(tc.tile_pool(name="ps_mm", bufs=2, space="PSUM"))
        wqkp = ctx.enter_context(tc.tile_pool(name="wqkp", bufs=4))
        wvp = ctx.enter_context(tc.tile_pool(name="wvp", bufs=2))
        wpp = ctx.enter_context(tc.tile_pool(name="wpp", bufs=2))

        # ---- resident tiles ----
        xT_sb = big.tile([P, CO, T], BF16, name="xT_sb")       # 16KB/part
        qkT = big.tile([P, 2 * C // P, T], BF16, name="qkT")   # 32KB/part
        v_pad = big.tile([P, TO, H, HS + 1], BF16, name="v_pad")  # 16.6KB
        outT = big.tile([P, CO, T], BF16, name="outT")         # 16KB/part
        bqk_sb = consts.tile([P, 2 * C // P], F32, name="bqk_sb")
        mask2_sb = consts.tile([P, 2, P], BF16, name="mask2_sb")
        bv_bc = consts.tile([P, C], F32, name="bv_bc")
        bproj_bc = consts.tile([P, C], F32, name="bproj_bc")

        xT_r = xT_d.rearrange("(co p) t -> p co t", p=P)
        wqk_r = wqk_d.rearrange("(co p) r -> p co r", p=P)
        wv_r = wv_d.rearrange("(co p) n -> p co n", p=P)
        wproj_r = wproj_d.rearrange("(co p) n -> p co n", p=P)
        y_r = y_d.rearrange("(tb p) c -> p tb c", p=P)

        # ---- startup DMAs: first weight block, then xT half 0 in co-pair
        # quarters so the first qkT matmuls start after ~256KB, not 1MB ----
        w_tiles = {}

        def dma_wm(m, split=False):
            t = wqkp.tile([P, CO, P], BF16, name=f"wm{m}", tag="wqk")
            if split:
                nc.sync.dma_start(t[:, 0:4, :], wqk_r[:, 0:4, m * P:(m + 1) * P])
                nc.sync.dma_start(t[:, 4:8, :], wqk_r[:, 4:8, m * P:(m + 1) * P])
            else:
                nc.sync.dma_start(t, wqk_r[:, :, m * P:(m + 1) * P])
            w_tiles[m] = t

        dma_wm(0, split=True)
        for q4 in range(4):
            nc.gpsimd.dma_start(xT_sb[:, 2 * q4:2 * q4 + 2, 0:512],
                                xT_r[:, 2 * q4:2 * q4 + 2, 0:512])
        dma_wm(NPAIR)
        nc.sync.dma_start(bqk_sb, bqk_d.rearrange("(m p) -> p m", p=P))
        for q4 in range(4):
            nc.gpsimd.dma_start(xT_sb[:, 2 * q4:2 * q4 + 2, 512:1024],
                                xT_r[:, 2 * q4:2 * q4 + 2, 512:1024])
        wv_tiles = [None, None]
        wv_tiles[0] = wvp.tile([P, CO, 512], BF16, name="wv0", tag="wv")
        nc.sync.dma_start(wv_tiles[0], wv_r[:, :, 0:512])
        rows = tc.alloc_tile_pool(name="rows", bufs=1)
        bv_row = rows.tile([1, C], F32, name="bv_row")
        nc.sync.dma_start(bv_row, bv_d[None, :])
        bproj_row = rows.tile([1, C], F32, name="bproj_row")
        nc.sync.dma_start(bproj_row, bproj_d[None, :])
        nc.sync.dma_start(mask2_sb, mask2_d.rearrange("p (h q) -> p h q", h=2))

        # bias broadcasts + ones column on the pool/vector engines
        nc.gpsimd.partition_broadcast(bv_bc, bv_row)
        nc.gpsimd.partition_broadcast(bproj_bc, bproj_row)
        rows.release()
        nc.vector.memset(v_pad[:, :, :, HS:HS + 1], 1.0)

        # ---- gemm unit emitters (each ~1.8us of PE work) ----
        def qkT_unit(m, n2):
            def go():
                ps = ps_mm.tile([P, 512], F32, name=f"qkps{m}_{n2}", tag="mm")
                for co in range(CO):
                    nc.tensor.matmul(
                        ps, w_tiles[m][:, co, :],
                        xT_sb[:, co, n2 * 512:(n2 + 1) * 512],
                        start=(co == 0), stop=(co == CO - 1))
                nc.vector.tensor_scalar_add(
                    qkT[:, m, n2 * 512:(n2 + 1) * 512], ps,
                    bqk_sb[:, m:m + 1])
            return go

        def v_unit(tb, n2):
            def go():
                ps = ps_mm.tile([P, 512], F32, name=f"vps{tb}_{n2}", tag="mm")
                for co in range(CO):
                    nc.tensor.matmul(
                        ps, xT_sb[:, co, tb * P:(tb + 1) * P],
                        wv_tiles[n2][:, co, :],
                        start=(co == 0), stop=(co == CO - 1))
                nc.vector.tensor_tensor(
                    out=v_pad[:, tb, n2 * 8:(n2 + 1) * 8, 0:HS],
                    in0=ps.rearrange("p (h d) -> p h d", d=HS),
                    in1=bv_bc[:, n2 * 512:(n2 + 1) * 512].rearrange(
                        "p (h d) -> p h d", d=HS),
                    op=mybir.AluOpType.add)
            return go

        # ---- attention-phase pools ----
        attn_ctx = ExitStack()
        ps_sc = attn_ctx.enter_context(
            tc.tile_pool(name="ps_sc", bufs=4, space="PSUM"))
        ps_pv = attn_ctx.enter_context(
            tc.tile_pool(name="ps_pv", bufs=2, space="PSUM"))
        ptp = attn_ctx.enter_context(tc.tile_pool(name="ptp", bufs=2))
        nrm = attn_ctx.enter_context(tc.tile_pool(name="nrm", bufs=2))

        def emit_pair(j, filler):
            """Scores + exp + mask + PV + normalize for head pair j.

            Emission order per qc: all score chunks (both heads, exp'd as
            they land), then PV for both heads. Filler gemm units are
            popped between chunks to cover ACT latency.
            """
            m_q, m_k = j, NPAIR + j
            pts = {}

            def pop():
                if filler:
                    filler.pop(0)()

            for qc in (0, 1):
                todo = [(kb, s, w) for kb in range(TO)
                        for (s, w) in CHUNKS[kb] if s // 512 == qc]
                n = 0
                for hh in (0, 1):
                    pb = hh * HS
                    for (kb, s, w) in todo:
                        ps = ps_sc.tile([P, 512], F32, name=f"sc{j}_{qc}",
                                        tag="sc")
                        nc.tensor.matmul(
                            ps[:, 0:w],
                            qkT[pb:pb + HS, m_k, kb * P:(kb + 1) * P],
                            qkT[pb:pb + HS, m_q, s:s + w],
                            start=True, stop=True, tile_position=(pb, 0))
                        if hh == 0:
                            pts[(kb, qc)] = ptp.tile(
                                [P, 2, w], BF16, name=f"pt{j}_{kb}_{qc}",
                                tag=f"pt{kb}_{qc}")
                        pt = pts[(kb, qc)]
                        nc.scalar.activation(pt[:, hh, 0:w], ps[:, 0:w],
                                             AF.Exp)
                        if hh == 1 and s == kb * P:
                            # diagonal block: causal mask for both heads
                            nc.vector.tensor_tensor(
                                out=pt[:, :, 0:P], in0=pt[:, :, 0:P],
                                in1=mask2_sb, op=mybir.AluOpType.mult)
                        if dbg is not None and j == 2 and hh == 1:
                            doff = sum(ww for kb2 in range(TO)
                                       for (ss, ww) in CHUNKS[kb2]
                                       if (kb2, ss) < (kb, s))
                            nc.sync.dma_start(
                                dbg["pt"][:, :, doff:doff + w], pt[:, :, 0:w])
                        n += 1
                        if n % 3 == 0:
                            pop()
                pop()
                for hh in (0, 1):
                    h = 2 * j + hh
                    pb = hh * HS
                    ps_o = ps_pv.tile([HS + 1, 512], F32, name=f"o{j}_{qc}",
                                      tag="pv")
                    for i, (kb, s, w) in enumerate(todo):
                        off = s - qc * 512
                        nc.tensor.matmul(
                            ps_o[:, off:off + w],
                            v_pad[:, kb, h, :],
                            pts[(kb, qc)][:, hh, 0:w],
                            start=(i == 0), stop=(i == len(todo) - 1))
                    srow = nrm.tile([1, 512], F32, name=f"sr{j}", tag="sr")
                    nc.vector.tensor_copy(srow, ps_o[HS:HS + 1, :])
                    rrow = nrm.tile([1, 512], F32, name=f"rr{j}", tag="rr")
                    nc.vector.reciprocal_approx_fast(rrow, srow)
                    brow = nrm.tile([HS, 512], F32, name=f"br{j}", tag="br")
                    nc.gpsimd.partition_broadcast(brow, rrow)
                    if dbg is not None and j == 2:
                        nc.sync.dma_start(dbg["brow"][:, hh, qc, :], brow)
                    nc.vector.tensor_mul(
                        outT[pb:pb + HS, j, qc * 512:(qc + 1) * 512],
                        ps_o[0:HS, :], brow)

        # ---- pre-loop gemm: qkT for pair 0, all of v half 0 ----
        for u in [qkT_unit(0, 0), qkT_unit(NPAIR, 0),
                  qkT_unit(0, 1), qkT_unit(NPAIR, 1)]:
            u()
        for tb in range(TO):
            v_unit(tb, 0)()

        wproj_sb = [None, None]
        filler = []
        for j in range(NPAIR):
            if j + 1 < NPAIR:
                dma_wm(j + 1)
                dma_wm(NPAIR + j + 1)
                filler += [qkT_unit(j + 1, 0), qkT_unit(NPAIR + j + 1, 0)]
                if j + 1 < NPAIR - 1:
                    filler += [qkT_unit(j + 1, 1), qkT_unit(NPAIR + j + 1, 1)]
            if j == NPAIR - 1:
                # deferred: pair 7's qc1 q/k blocks double as its only filler
                filler += [qkT_unit(j, 1), qkT_unit(NPAIR + j, 1)]
            if j == 1:
                wv_tiles[1] = wvp.tile([P, CO, 512], BF16, name="wv1",
                                       tag="wv")
                nc.sync.dma_start(wv_tiles[1], wv_r[:, :, 512:1024])
            if j == 2:
                filler += [v_unit(tb, 1) for tb in range(4)]
            if j == 3:
                filler += [v_unit(tb, 1) for tb in range(4, TO)]
            if j in (5, 6):
                n2 = j - 5
                wproj_sb[n2] = wpp.tile([P, CO, 512], BF16,
                                        name=f"wproj{n2}", tag="wproj")
                nc.sync.dma_start(wproj_sb[n2],
                                  wproj_r[:, :, n2 * 512:(n2 + 1) * 512])
            emit_pair(j, filler)
            while filler:
                filler.pop(0)()

        if dbg is not None:
            nc.sync.dma_start(dbg["qkT"], qkT)
            nc.sync.dma_start(dbg["vpad"], v_pad)
            nc.sync.dma_start(dbg["outT"], outT)

        # ---- output projection ----
        attn_ctx.close()
        with tc.tile_pool(name="ypool", bufs=3) as yp:
            for n2 in range(C // 512):
                for tb in range(TO):
                    ps = ps_mm.tile([P, 512], F32, name=f"y_ps{tb}_{n2}",
                                    tag="mm")
                    for co in range(CO):
                        nc.tensor.matmul(
                            ps, outT[:, co, tb * P:(tb + 1) * P],
                            wproj_sb[n2][:, co, :],
                            start=(co == 0), stop=(co == CO - 1))
                    y_sb = yp.tile([P, 512], F32, name=f"y_sb{tb}_{n2}",
                                   tag="y")
                    nc.vector.tensor_add(y_sb, ps,
                                         bproj_bc[:, n2 * 512:(n2 + 1) * 512])
                    nc.sync.dma_start(
                        y_r[:, tb, n2 * 512:(n2 + 1) * 512], y_sb)


_NC_CACHE = None


def _get_nc():
    global _NC_CACHE
    if _NC_CACHE is None:
        _NC_CACHE = build_nc()
    return _NC_CACHE


def kernel(x, W_qkv, b_qkv, W_proj, b_proj):
    """Full-input entry point: shards batch across 8 cores, returns [B,T,C]."""
    global LAST_RESULTS
    bf16 = ml_dtypes.bfloat16
    x = np.asarray(x, dtype=np.float32)
    W_qkv = np.asarray(W_qkv, dtype=np.float32)
    b_qkv = np.asarray(b_qkv, dtype=np.float32)
    W_proj = np.asarray(W_proj, dtype=np.float32)
    b_proj = np.asarray(b_proj, dtype=np.float32)

    scale = 1.0 / np.sqrt(HS)
    wqk = W_qkv[:, :2 * C].copy()
    wqk[:, :C] *= scale
    bqk = b_qkv[:2 * C].copy()
    bqk[:C] *= scale
    wv = np.ascontiguousarray(W_qkv[:, 2 * C:])
    bv = np.ascontiguousarray(b_qkv[2 * C:])
    # mask[k, q] = 1 where q >= k (valid, causal), else 0; doubled for the
    # paired pt tiles [P, 2, P]
    m1 = np.triu(np.ones((P, P), dtype=np.float32))
    mask2 = np.stack([m1, m1], axis=1).reshape(P, 2 * P).astype(bf16)

    common = dict(wqk=wqk.astype(bf16), wv=wv.astype(bf16),
                  wproj=W_proj.astype(bf16), bqk=bqk, bv=bv,
                  bproj=b_proj, mask2=mask2)
    in_maps = [dict(xT=x[b].T.astype(bf16), **common) for b in range(B)]

    nc = _get_nc()
    res = run_bass_kernel_spmd(nc, in_maps, core_ids=list(range(B)),
                               trace=TRACE)
    LAST_RESULTS = res
    y = np.stack([res.results[b]["y"] for b in range(B)], axis=0)
    return y
